# revision 23
# baseline (speedup 1.0000x reference)
"""Distributed Trainium2 Bass kernel for nn_ArchetipesNetwork.

Recurrent network: T=256 steps, N=64 archetype modules (each with its own
512x512 W_h / W_x), coupled per-step through a 64x64 connection matrix.

Sharding: archetype axis N split over 8 NeuronCores (8 archetypes/core).
Per step: AllGather of the 8x512 local hy outputs -> full 64x512 outs,
connection-mix + input gating, batched per-archetype matvecs on the
TensorEngine (h stationary / W^T streaming, 4-way column-tiled PE array),
RON-cell elementwise update.

Layout notes (per core, L=8 local archetypes, H=512, chunks c=0..3 of 128):
 - "T layout" [128, 32]: partition p = i (within chunk), col = c*8 + n.
   Used for states hyT/hzT, gated input insT, bias bT, x^T, tanh.
 - "natural" [8, 512]: partition n, free i. Used for AllGather / outputs.
 - W^T in SBUF: [128, L*4*512], block (n,c) holds W[n][:, c*128:+128]^T so
   matvec = sum_c (h-chunk-c stationary).T @ (W^T chunk-c streaming).
 - All matmuls use PE tile_size (128,32) (column-tiled 4x), tile q = n%4.
All host-side transposes of constant tensors (W, x, b, C, identities) are
plain numpy done during sharding.
"""

import sys

sys.path.insert(0, "/opt/trn_rl_repo")

import numpy as np

from concourse import bacc, bass, mybir, tile
from concourse.bass_utils import run_bass_kernel_spmd

F32 = mybir.dt.float32

T, N, H = 256, 64, 512
R = 8                 # cores
L = N // R            # local archetypes per core
NCH = H // 128        # 4 chunks of the hidden dim
DT_ = 0.01
AF = mybir.ActivationFunctionType
ALU = mybir.AluOpType


def build_graph(t_steps: int = T) -> bass.Bass:
    nc = bacc.Bacc(num_devices=R)

    # ---- parameters (per-core shards; all transposes done on host) ----
    x_nat = nc.declare_dram_parameter("x_nat", [t_steps, L, H], F32, isOutput=False)
    x_t = nc.declare_dram_parameter("x_t", [t_steps, 128, NCH * L], F32, isOutput=False)
    whT = nc.declare_dram_parameter("whT", [L, NCH, 128, H], F32, isOutput=False)
    wxT = nc.declare_dram_parameter("wxT", [L, NCH, 128, H], F32, isOutput=False)
    bT_p = nc.declare_dram_parameter("bT", [128, NCH * L], F32, isOutput=False)
    cT_p = nc.declare_dram_parameter("cT", [128, L], F32, isOutput=False)
    i128_p = nc.declare_dram_parameter("i128", [128, 128], F32, isOutput=False)
    i8s_p = nc.declare_dram_parameter("i8s", [128, L], F32, isOutput=False)
    hyT0_p = nc.declare_dram_parameter("hyT0", [128, NCH * L], F32, isOutput=False)
    hzT0_p = nc.declare_dram_parameter("hzT0", [128, NCH * L], F32, isOutput=False)
    out_states = nc.declare_dram_parameter(
        "out_states", [t_steps, L, 2, H], F32, isOutput=True
    )
    out_ins = nc.declare_dram_parameter("out_ins", [t_steps, L, H], F32, isOutput=True)

    groups = [list(range(R))]

    with tile.TileContext(nc) as tc:
        with (
            tc.tile_pool(name="persist", bufs=1) as pp,
            tc.tile_pool(name="work", bufs=2) as wp,
            tc.tile_pool(name="psum", bufs=1, space="PSUM") as psp,
            tc.tile_pool(name="dram", bufs=2, space="DRAM") as dp,
        ):
            # ---- persistent SBUF ----
            whT_sb = pp.tile([128, L * NCH * H], F32)
            wxT_sb = pp.tile([128, L * NCH * H], F32)
            cT_sb = pp.tile([128, L], F32)
            i128_sb = pp.tile([128, 128], F32)
            i8s_sb = pp.tile([128, L], F32)
            bT_sb = pp.tile([128, NCH * L], F32)
            hyT = pp.tile([128, NCH * L], F32)
            hzT = pp.tile([128, NCH * L], F32)
            # padded stationary layouts: per chunk c, 16 cols =
            # [h0,0, h1,0, h2,0, h3,0, 0,h4, 0,h5, 0,h6, 0,h7]
            # live col for arch n (q=n%4, w=n//4): c*16 + 2q + 9w
            hyT2 = pp.tile([128, NCH * 16], F32)
            insT2 = pp.tile([128, NCH * 16], F32)
            outs_full = pp.tile([128, H], F32)     # rows 0:64 gathered, 64:128 zero
            preN_fat = pp.tile([128, H], F32)      # rows {32q+w} live, rest zero

            # one-time loads
            for n in range(L):
                for c in range(NCH):
                    nc.sync.dma_start(
                        out=whT_sb[:, (n * NCH + c) * H:(n * NCH + c + 1) * H],
                        in_=whT[n, c],
                    )
                    nc.sync.dma_start(
                        out=wxT_sb[:, (n * NCH + c) * H:(n * NCH + c + 1) * H],
                        in_=wxT[n, c],
                    )
            nc.sync.dma_start(out=cT_sb[:], in_=cT_p[:])
            nc.sync.dma_start(out=i128_sb[:], in_=i128_p[:])
            nc.sync.dma_start(out=i8s_sb[:], in_=i8s_p[:])
            nc.sync.dma_start(out=bT_sb[:], in_=bT_p[:])
            nc.sync.dma_start(out=hyT[:], in_=hyT0_p[:])
            nc.sync.dma_start(out=hzT[:], in_=hzT0_p[:])
            nc.vector.memset(outs_full[:], 0.0)
            nc.vector.memset(preN_fat[:], 0.0)
            nc.vector.memset(hyT2[:], 0.0)
            nc.vector.memset(insT2[:], 0.0)

            def pad_copy(dst2, srcT, w):
                # srcT cols (c*8 + 4w + q) -> dst2 live cols (c*16 + 2q + 9w)
                nc.vector.tensor_copy(
                    dst2.rearrange("p (c x) -> p c x", x=16)[:, :, 9 * w:9 * w + 7:2],
                    srcT.rearrange("p (c d) -> p c d", d=L)[:, :, 4 * w:4 * w + 4],
                )

            pad_copy(hyT2, hyT, 0)
            pad_copy(hyT2, hyT, 1)

            # stg holds the natural-layout (hy | hz) of the current step;
            # its hy half feeds the next step's AllGather. outs at t=0 are
            # zeros per the reference (init_outs = zeros), independent of
            # initial_states.
            stg = wp.tile([128, 2 * H], F32, tag="stg", bufs=2)
            nc.vector.memset(stg[0:L, :], 0.0)

            for t in range(t_steps):
                # ---- x loads (prefetchable) ----
                xT_sb = wp.tile([128, NCH * L], F32, tag="xT", bufs=3)
                nc.sync.dma_start(out=xT_sb[:], in_=x_t[t])
                xN_sb = wp.tile([128, H], F32, tag="xN", bufs=3)
                nc.sync.dma_start(out=xN_sb[0:L, :], in_=x_nat[t])

                # ---- AllGather of previous outs (stg hy half) ----
                cc_in = dp.tile([L, H], F32, tag="cc_in")
                cc_out = dp.tile([N, H], F32, tag="cc_out", addr_space="Shared")
                nc.sync.dma_start(out=cc_in[:], in_=stg[0:L, 0:H])
                nc.gpsimd.collective_compute(
                    "AllGather",
                    ALU.bypass,
                    replica_groups=groups,
                    ins=[cc_in[:].opt()],
                    outs=[cc_out[:].opt()],
                )
                nc.sync.dma_start(out=outs_full[0:64, :], in_=cc_out[:])

                # ---- W_h matvecs (do not need the gather) ----
                # lhsT is a 2-col zero-padded pair so both waves of col-tile q
                # accumulate into adjacent psum rows (32q, 32q+1) of one bank.
                pre_ps = psp.tile([128, H], F32, tag="pre")
                for c in range(NCH):
                    for q in range(4):
                        for w in range(2):
                            n = 4 * w + q
                            nc.tensor.matmul(
                                out=pre_ps[32 * q:32 * q + 2, 0:H],
                                lhsT=hyT2[:, c * 16 + 2 * q + 8 * w:
                                          c * 16 + 2 * q + 8 * w + 2],
                                rhs=whT_sb[:, (n * NCH + c) * H:(n * NCH + c + 1) * H],
                                start=(c == 0 and w == 0),
                                stop=False,
                                tile_position=(0, 32 * q),
                            )

                # ---- connection mix, transposed: insT = (C @ outs)^T ----
                insT_ps = psp.tile([128, H], F32, tag="insT")
                for k in range(4):
                    for c in range(NCH):
                        nc.tensor.matmul(
                            out=insT_ps[32 * k:32 * (k + 1), c * L:(c + 1) * L],
                            lhsT=outs_full[:, c * 128 + 32 * k:c * 128 + 32 * (k + 1)],
                            rhs=cT_sb[:, 0:L],
                            start=(c == 0),
                            stop=(c == NCH - 1),
                            tile_position=(0, 32 * k),
                        )
                # gate with x^T, writing straight into the padded live cols
                for w in range(2):
                    nc.vector.tensor_mul(
                        insT2.rearrange("p (c x) -> p c x", x=16)[:, :, 9 * w:9 * w + 7:2],
                        insT_ps.rearrange("p (c d) -> p c d", d=L)[:, 0:NCH, 4 * w:4 * w + 4],
                        xT_sb.rearrange("p (c d) -> p c d", d=L)[:, :, 4 * w:4 * w + 4],
                    )

                # ---- connection mix, natural: insN = C_loc @ outs ----
                insN_ps = psp.tile([128, H], F32, tag="insN")
                nc.tensor.matmul(
                    out=insN_ps[0:L, :],
                    lhsT=cT_sb[:, 0:L],
                    rhs=outs_full[:, 0:H],
                    start=True,
                    stop=True,
                    tile_position=(0, 0),
                )
                insN_sb = wp.tile([128, H], F32, tag="insNsb")
                nc.vector.tensor_mul(insN_sb[0:L, :], insN_ps[0:L, :], xN_sb[0:L, :])
                nc.sync.dma_start(out=out_ins[t], in_=insN_sb[0:L, :])

                # ---- W_x matvecs (accumulate into pre) ----
                for c in range(NCH):
                    for q in range(4):
                        for w in range(2):
                            n = 4 * w + q
                            nc.tensor.matmul(
                                out=pre_ps[32 * q:32 * q + 2, 0:H],
                                lhsT=insT2[:, c * 16 + 2 * q + 8 * w:
                                           c * 16 + 2 * q + 8 * w + 2],
                                rhs=wxT_sb[:, (n * NCH + c) * H:(n * NCH + c + 1) * H],
                                start=False,
                                stop=(c == NCH - 1 and w == 1),
                                tile_position=(0, 32 * q),
                            )

                # ---- evacuate pre row-pairs to SBUF ----
                for q in range(4):
                    eng = nc.vector.tensor_copy if q < 2 else nc.scalar.copy
                    eng(
                        preN_fat[32 * q:32 * q + 2, 0:H],
                        pre_ps[32 * q:32 * q + 2, 0:H],
                    )

                # ---- transpose pre into T layout via identity matmuls ----
                preT_ps = psp.tile([128, H], F32, tag="preT")
                for k in range(4):
                    for c in range(NCH):
                        nc.tensor.matmul(
                            out=preT_ps[32 * k:32 * (k + 1), c * L:(c + 1) * L],
                            lhsT=preN_fat[:, c * 128 + 32 * k:c * 128 + 32 * (k + 1)],
                            rhs=i8s_sb[:, 0:L],
                            start=(c == 0),
                            stop=(c == NCH - 1),
                            tile_position=(0, 32 * k),
                        )

                # ---- bias + tanh + RON update (T layout) ----
                u_sb = wp.tile([128, NCH * L], F32, tag="u")
                nc.vector.tensor_add(u_sb[:], preT_ps[:, 0:NCH * L], bT_sb[:])
                tanhT = wp.tile([128, NCH * L], F32, tag="tanh")
                nc.scalar.activation(tanhT[:], u_sb[:], AF.Tanh)
                u1 = wp.tile([128, NCH * L], F32, tag="u1")
                nc.vector.tensor_sub(u1[:], tanhT[:], hyT[:])
                u2 = wp.tile([128, NCH * L], F32, tag="u2")
                nc.vector.tensor_sub(u2[:], u1[:], hzT[:])
                nc.vector.scalar_tensor_tensor(
                    hzT[:], u2[:], DT_, hzT[:], op0=ALU.mult, op1=ALU.add
                )
                nc.vector.scalar_tensor_tensor(
                    hyT[:], hzT[:], DT_, hyT[:], op0=ALU.mult, op1=ALU.add
                )
                pad_copy(hyT2, hyT, 0)
                pad_copy(hyT2, hyT, 1)

                # ---- transpose states back to natural layout ----
                backT_ps = psp.tile([128, 2 * H], F32, tag="backT")
                for c in range(NCH):
                    nc.tensor.matmul(
                        out=backT_ps[0:L, c * 128:(c + 1) * 128],
                        lhsT=hyT[:, c * L:(c + 1) * L],
                        rhs=i128_sb[:, 0:128],
                        start=(c == 0),
                        stop=(c == NCH - 1),
                        tile_position=(0, 0),
                    )
                for c in range(NCH):
                    nc.tensor.matmul(
                        out=backT_ps[0:L, H + c * 128:H + (c + 1) * 128],
                        lhsT=hzT[:, c * L:(c + 1) * L],
                        rhs=i128_sb[:, 0:128],
                        start=(c == 0),
                        stop=(c == NCH - 1),
                        tile_position=(0, 0),
                    )
                stg = wp.tile([128, 2 * H], F32, tag="stg", bufs=2)
                nc.vector.tensor_copy(stg[0:L, 0:H], backT_ps[0:L, 0:H])
                nc.scalar.copy(stg[0:L, H:2 * H], backT_ps[0:L, H:2 * H])
                nc.sync.dma_start(
                    out=out_states[t].rearrange("l s h -> l (s h)"),
                    in_=stg[0:L, :],
                )

    nc.finalize()
    return nc


def shard_inputs(x, connection_matrix, W_h, W_x, b, initial_states, t_steps=T):
    """Split + re-layout the full inputs into 8 per-core in_maps (pure numpy)."""
    x = np.ascontiguousarray(np.asarray(x, np.float32)[:t_steps])
    cm = np.asarray(connection_matrix, np.float32)
    W_h = np.asarray(W_h, np.float32)
    W_x = np.asarray(W_x, np.float32)
    b = np.asarray(b, np.float32)
    init = np.asarray(initial_states, np.float32)

    i128 = np.eye(128, dtype=np.float32)
    i8s = np.zeros((128, L), np.float32)
    for n in range(L):
        i8s[32 * (n % 4) + n // 4, n] = 1.0

    def tlay(a):  # (L, H) -> [128, 32] T layout, col = c*8 + n
        return np.ascontiguousarray(
            a.reshape(L, NCH, 128).transpose(2, 1, 0).reshape(128, NCH * L)
        )

    in_maps = []
    for r in range(R):
        sl = slice(r * L, (r + 1) * L)
        x_loc = x[:, sl]  # (T, L, H)
        xT = np.ascontiguousarray(
            x_loc.reshape(t_steps, L, NCH, 128).transpose(0, 3, 2, 1)
            .reshape(t_steps, 128, NCH * L)
        )
        cT = np.zeros((128, L), np.float32)
        cT[:N, :] = cm[sl].T
        in_maps.append({
            "x_nat": np.ascontiguousarray(x_loc),
            "x_t": xT,
            "whT": np.ascontiguousarray(
                W_h[sl].transpose(0, 2, 1).reshape(L, NCH, 128, H)
            ),
            "wxT": np.ascontiguousarray(
                W_x[sl].transpose(0, 2, 1).reshape(L, NCH, 128, H)
            ),
            "bT": tlay(b[sl]),
            "cT": cT,
            "i128": i128,
            "i8s": i8s,
            "hyT0": tlay(init[sl, 0]),
            "hzT0": tlay(init[sl, 1]),
        })
    return in_maps


def assemble_outputs(results, x, initial_states, t_steps=T):
    states_all = np.zeros((t_steps + 1, N, 2, H), np.float32)
    ins_all = np.zeros((t_steps + 1, N, H), np.float32)
    states_all[0] = np.asarray(initial_states, np.float32)
    ins_all[0] = np.asarray(x, np.float32)[0]
    for r in range(R):
        sl = slice(r * L, (r + 1) * L)
        states_all[1:, sl] = results[r]["out_states"]
        ins_all[1:, sl] = results[r]["out_ins"]
    return states_all, ins_all


def run(inputs, t_steps=T, trace=False):
    nc = build_graph(t_steps)
    in_maps = shard_inputs(**inputs, t_steps=t_steps)
    res = run_bass_kernel_spmd(nc, in_maps, core_ids=list(range(R)), trace=trace)
    out = assemble_outputs(
        res.results, inputs["x"], inputs["initial_states"], t_steps=t_steps
    )
    return out, res


def kernel(**inputs):
    (states_all, ins_all), _ = run(inputs, t_steps=T, trace=False)
    return states_all, ins_all
